# revision 11
# baseline (speedup 1.0000x reference)
"""Trainium2 Bass kernel for the difflogic LogicLayer problem.

Math: out[b,n] = sum_g softmax(w[n])_g * gate_g(a,b) with a = x[b, idx_a[n]],
b = x[b, idx_b[n]].  Every 2-input soft gate is linear in {1, a, b, ab}, so
the whole layer collapses to

    out[b,n] = c0[n] + c1[n]*a + c2[n]*b + c3[n]*(a*b)

with 4 per-neuron coefficients folded from the softmaxed weights (host-side
weight preprocessing, like batchnorm folding).  The device kernel is pure
data movement + 4 elementwise ops:

Sharding: OUT is sharded 8 ways (2048 neurons/core); every core keeps the
full batch, so a gathered "row" of x^T is 2048*4 = 8 KiB contiguous — only
4096 gather descriptors per core.  x is fed transposed (x^T [16384, 2048])
and replicated; per-core HBM traffic = 32 MiB gather reads + 16 MiB store.

Per core, per tile of 256 neurons (8 tiles, double buffered):
  gpsimd.dma_gather a-rows, b-rows   [128p, 2, 2048] from HBM
  ACT: tmp = c3*a + c2 ; o = c1*a + c0   (per-partition scalar APs)
  DVE: tmp *= b ; o += tmp
  HWDGE store o -> out[t]
"""

import os
import sys

import numpy as np

sys.path.insert(0, "/opt/trn_rl_repo")

B = 2048
IN_DIM = 16384
OUT_DIM = 16384
N_CORES = 8

OUT_PER_CORE = OUT_DIM // N_CORES  # 2048
PART = 128
CHUNK = 2                          # chunks per tile (128 idx each)
TILE_IDX = PART * CHUNK            # 256 neurons per gather/tile
NT = OUT_PER_CORE // TILE_IDX      # 8 tiles per core

LAST_EXEC_TIME_NS = None
LAST_RESULTS = None

# difflogic gate -> (1, a, b, ab) basis coefficients, gate order as in the
# reference _gates().
_GATE_BASIS = np.array(
    [
        # const  a    b    ab
        [0, 0, 0, 0],    # FALSE
        [0, 0, 0, 1],    # a AND b
        [0, 1, 0, -1],   # a AND NOT b
        [0, 1, 0, 0],    # a
        [0, 0, 1, -1],   # NOT a AND b
        [0, 0, 1, 0],    # b
        [0, 1, 1, -2],   # XOR
        [0, 1, 1, -1],   # OR
        [1, -1, -1, 1],  # NOR
        [1, -1, -1, 2],  # XNOR
        [1, 0, -1, 0],   # NOT b
        [1, 0, -1, 1],   # a OR NOT b
        [1, -1, 0, 0],   # NOT a
        [1, -1, 0, 1],   # NOT a OR b
        [1, 0, 0, -1],   # NAND
        [1, 0, 0, 0],    # TRUE
    ],
    dtype=np.float64,
)


def _coeffs_from_weights(weights: np.ndarray) -> np.ndarray:
    """softmax(weights) projected onto the (1, a, b, ab) basis -> [OUT, 4] f32."""
    w = weights.astype(np.float64)
    w = w - w.max(axis=-1, keepdims=True)
    p = np.exp(w)
    p /= p.sum(axis=-1, keepdims=True)
    return (p @ _GATE_BASIS).astype(np.float32)


_NC_CACHE = {}


def _build_bass(n_rows, elem, nt, chunk=CHUNK, part=PART, n_rep=1, reset_sems=False):
    """One-core SPMD program; all cores run the same code on different inputs.

    n_rep > 1 re-runs the whole tile loop (same inputs/outputs) for steady-state
    HW timing: slope between n_rep values isolates per-rep exec time."""
    import concourse.bacc as bacc
    import concourse.mybir as mybir
    from concourse.library_config import mlp

    tile_idx = part * chunk
    idx_cols = tile_idx // 16  # int16 gather-index columns per tile

    nc = bacc.Bacc("TRN2")
    xt = nc.dram_tensor("xt", [n_rows, elem], mybir.dt.float32, kind="ExternalInput")
    idxa = nc.dram_tensor("idxa", [part, nt * idx_cols], mybir.dt.int16, kind="ExternalInput")
    idxb = nc.dram_tensor("idxb", [part, nt * idx_cols], mybir.dt.int16, kind="ExternalInput")
    coef = nc.dram_tensor("coef", [part, nt * chunk * 4], mybir.dt.float32, kind="ExternalInput")
    out = nc.dram_tensor("out", [nt, part, chunk, elem], mybir.dt.float32, kind="ExternalOutput")

    f32 = mybir.dt.float32
    Copy = mybir.ActivationFunctionType.Identity

    from contextlib import ExitStack
    with ExitStack() as _stack:
        ec = _stack.enter_context
        idxa_s = ec(nc.sbuf_tensor("idxa_s", [part, nt * idx_cols], mybir.dt.int16))
        idxb_s = ec(nc.sbuf_tensor("idxb_s", [part, nt * idx_cols], mybir.dt.int16))
        coef_s = ec(nc.sbuf_tensor("coef_s", [part, nt * chunk * 4], f32))
        a0 = ec(nc.sbuf_tensor("a0", [part, chunk, elem], f32))
        a1 = ec(nc.sbuf_tensor("a1", [part, chunk, elem], f32))
        b0 = ec(nc.sbuf_tensor("b0", [part, chunk, elem], f32))
        b1 = ec(nc.sbuf_tensor("b1", [part, chunk, elem], f32))
        t0 = ec(nc.sbuf_tensor("t0", [part, chunk, elem], f32))
        t1 = ec(nc.sbuf_tensor("t1", [part, chunk, elem], f32))
        o0 = ec(nc.sbuf_tensor("o0", [part, chunk, elem], f32))
        o1 = ec(nc.sbuf_tensor("o1", [part, chunk, elem], f32))
        ld_sem = ec(nc.semaphore("ld"))
        ga0_sem = ec(nc.semaphore("ga0"))
        ga1_sem = ec(nc.semaphore("ga1"))
        gb0_sem = ec(nc.semaphore("gb0"))
        gb1_sem = ec(nc.semaphore("gb1"))
        dve_sem = ec(nc.semaphore("dve"))
        st0_sem = ec(nc.semaphore("st0"))
        st1_sem = ec(nc.semaphore("st1"))
        fin_sem = ec(nc.semaphore("fin"))
        block = ec(nc.Block())
        a_bufs, b_bufs = [a0, a1], [b0, b1]
        t_bufs, o_bufs = [t0, t1], [o0, o1]
        ga_sems, gb_sems = [ga0_sem, ga1_sem], [gb0_sem, gb1_sem]
        st_sems = [st0_sem, st1_sem]

        @block.sync
        def _(sync):
            sync.dma_start(idxa_s[:, :], idxa[:, :]).then_inc(ld_sem, 16)
            sync.dma_start(idxb_s[:, :], idxb[:, :]).then_inc(ld_sem, 16)
            sync.dma_start(coef_s[:, :], coef[:, :]).then_inc(ld_sem, 16)
            for rep in range(n_rep):
                for t in range(nt):
                    T = rep * nt + t
                    j = T % 2
                    sync.wait_ge(dve_sem, 6 * T + 6)
                    sync.dma_start(out[t, :, :, :], o_bufs[j][:, :, :]).then_inc(st_sems[j], 16)
            ntot = nt * n_rep
            sync.wait_ge(st0_sem, 16 * ((ntot + 1) // 2))
            sync.wait_ge(st1_sem, 16 * (ntot // 2))
            if reset_sems:
                sync.sem_inc(fin_sem, 1)

        @block.gpsimd
        def _(gpsimd):
            gpsimd.load_library(mlp)
            gpsimd.wait_ge(ld_sem, 48)
            for rep in range(n_rep):
                for t in range(nt):
                    T = rep * nt + t
                    j = T % 2
                    if T >= 2:
                        gpsimd.wait_ge(dve_sem, 6 * T - 7)
                    gpsimd.dma_gather(
                        a_bufs[j][:, :, :],
                        xt[:, :],
                        idxa_s[:, t * idx_cols:(t + 1) * idx_cols],
                        tile_idx,
                        tile_idx,
                        elem,
                    ).then_inc(ga_sems[j], 16)
                    if T >= 2:
                        gpsimd.wait_ge(dve_sem, 6 * T - 9)
                    gpsimd.dma_gather(
                        b_bufs[j][:, :, :],
                        xt[:, :],
                        idxb_s[:, t * idx_cols:(t + 1) * idx_cols],
                        tile_idx,
                        tile_idx,
                        elem,
                    ).then_inc(gb_sems[j], 16)
            # end-of-program sem reset so the NEFF is safely re-executable:
            # sems otherwise persist across nrt executions.  (Disabled for
            # CoreSim runs: non-monotonic sem values confuse the race
            # detector; executions are serialized on HW so this is safe.)
            if reset_sems:
                gpsimd.wait_ge(fin_sem, 1)
                gpsimd.wait_ge(dve_sem, 6 * nt * n_rep)
                for s in (ld_sem, ga0_sem, ga1_sem, gb0_sem, gb1_sem,
                          dve_sem, st0_sem, st1_sem, fin_sem):
                    gpsimd.sem_clear(s)

        @block.vector
        def _(vector):
            mult = mybir.AluOpType.mult
            add = mybir.AluOpType.add
            for rep in range(n_rep):
                for t in range(nt):
                    T = rep * nt + t
                    j = T % 2
                    vector.wait_ge(ga_sems[j], 16 * (T // 2 + 1))
                    if T >= 2:
                        vector.wait_ge(st_sems[j], 16 * (T // 2))
                    base = t * chunk * 4
                    for c in range(chunk):
                        col = base + c * 4
                        vector.tensor_scalar(
                            t_bufs[j][:, c, :], a_bufs[j][:, c, :],
                            coef_s[:, col + 3:col + 4], coef_s[:, col + 2:col + 3],
                            mult, add,
                        ).then_inc(dve_sem, 1)
                    vector.wait_ge(gb_sems[j], 16 * (T // 2 + 1))
                    vector.wait_ge(dve_sem, 6 * T + 2)
                    vector.tensor_mul(
                        t_bufs[j][:, :, :], t_bufs[j][:, :, :], b_bufs[j][:, :, :]
                    ).then_inc(dve_sem, 1)
                    for c in range(chunk):
                        col = base + c * 4
                        vector.tensor_scalar(
                            o_bufs[j][:, c, :], a_bufs[j][:, c, :],
                            coef_s[:, col + 1:col + 2], coef_s[:, col + 0:col + 1],
                            mult, add,
                        ).then_inc(dve_sem, 1)
                    vector.wait_ge(dve_sem, 6 * T + 5)
                    vector.tensor_add(
                        o_bufs[j][:, :, :], o_bufs[j][:, :, :], t_bufs[j][:, :, :]
                    ).then_inc(dve_sem, 1)

    nc.compile()
    return nc


def _pack_idx(idx: np.ndarray, nt: int, tile_idx: int) -> np.ndarray:
    """Pack per-core indices into the dma_gather SBUF layout:
    tile t's index i lives at [i % 16, t*idx_cols + i//16] (int16)."""
    idx_cols = tile_idx // 16
    packed = np.zeros((PART, nt * idx_cols), dtype=np.int16)
    v = idx.astype(np.int16).reshape(nt, idx_cols, 16)  # [t, s, p], i = s*16+p
    block16 = v.transpose(2, 0, 1).reshape(16, nt * idx_cols)
    packed[:, :] = np.tile(block16, (PART // 16, 1))  # replicated per Q7 core
    return packed


def _pack_coef(cc: np.ndarray, nt: int, chunk: int) -> np.ndarray:
    """cc: [OUT_PER_CORE, 4] -> [128, nt*chunk*4]; neuron t*chunk*128 + c*128 + p
    lands at [p, (t*chunk + c)*4 + k]."""
    return np.ascontiguousarray(
        cc.reshape(nt, chunk, PART, 4).transpose(2, 0, 1, 3).reshape(PART, nt * chunk * 4)
    )


def prepare(inputs, n_rep=1):
    """Build (nc, in_maps) for the SPMD run."""
    x = np.asarray(inputs["x"], dtype=np.float32)
    weights = np.asarray(inputs["weights"], dtype=np.float32)
    idx_a = np.asarray(inputs["idx_a"])
    idx_b = np.asarray(inputs["idx_b"])

    cc = _coeffs_from_weights(weights)          # [OUT, 4]
    xt = np.ascontiguousarray(x.T)              # [IN, B] — 8 KiB rows

    key = (IN_DIM, B, NT, n_rep)
    if key not in _NC_CACHE:
        _NC_CACHE[key] = _build_bass(IN_DIM, B, NT, n_rep=n_rep)
    nc = _NC_CACHE[key]

    in_maps = []
    for c in range(N_CORES):
        n0 = c * OUT_PER_CORE
        n1 = n0 + OUT_PER_CORE
        in_maps.append({
            "xt": xt,
            "idxa": _pack_idx(idx_a[n0:n1], NT, TILE_IDX),
            "idxb": _pack_idx(idx_b[n0:n1], NT, TILE_IDX),
            "coef": _pack_coef(cc[n0:n1], NT, CHUNK),
        })
    return nc, in_maps


def assemble(results):
    """results: per-core dicts with 'out' [NT, 128, CHUNK, B] -> full [B, OUT] f32."""
    outs = []
    for c in range(N_CORES):
        o = results[c]["out"]
        outs.append(o.transpose(0, 2, 1, 3).reshape(OUT_PER_CORE, B))
    full = np.concatenate(outs, axis=0)  # [OUT, B]
    return np.ascontiguousarray(full.T).astype(np.float32, copy=False)


def kernel(x, weights, idx_a, idx_b):
    global LAST_EXEC_TIME_NS, LAST_RESULTS
    from concourse.bass_utils import run_bass_kernel_spmd

    nc, in_maps = prepare(
        {"x": x, "weights": weights, "idx_a": idx_a, "idx_b": idx_b}
    )
    res = run_bass_kernel_spmd(nc, in_maps, list(range(N_CORES)))
    LAST_EXEC_TIME_NS = res.exec_time_ns
    LAST_RESULTS = res
    return assemble(res.results)

